# revision 11
# speedup vs baseline: 1.0646x; 1.0018x over previous
"""Trainium2 Bass kernel for nn_Block_84155589198355 (dense transformer block).

Data-parallel B=8 over 8 cores; fp8 DoubleRow matmuls everywhere.

v2 changes vs baseline:
  - qt-granular causal attention: S/exp computed only on visible (qt, kt)
    tiles; no masked-region exp, no e-zeroing memsets, no tri on dead tiles.
  - V stored unpadded [128, t, h, 64]; AV emits per-head y_ps [64, 512]
    (DR dst must be partition-base 0). Softmax denominators come from
    dedicated ones-stationary DR matmuls with 64-row replicated output
    (PE cost = moving width only), so the reciprocal+normalize is one
    [64,512] DVE pair per (head, qc) instead of [1,512] ops + partition
    broadcasts. Odd heads (po=64) are DMA-shifted into y_sb[64:128].
  - MLP1 half-0 is interleaved into the qc1 attention sweep: the sweep is
    ACT-bound (exp) while PE idles; act-table swaps are free here.
  - exp of causal tiles is packed: one ACT instruction covers up to 4
    kt-tiles of a (h, qt) group.

Layouts (per core, T=1024, C=1024, H=16, D=64):
  - Q/K feature-major [128, NKC, T] fp8, head pair per chunk (po=(h%2)*64).
  - S cond (kt 0,1): [128k, 512q] stride-0-broadcast DR (2*K^T Q, exp scale
    0.0625); causal: [128k, 128q] per (h, qt, kt), diag tile gets tri add.
  - V token-major [128, t, h, 64] fp8 unpadded.
  - MLP1/MLP2 identical to baseline (3-term hi/lo fp8 splits).
"""

import sys

if "/opt/trn_rl_repo" not in sys.path:
    sys.path.insert(0, "/opt/trn_rl_repo")

import numpy as np
import ml_dtypes

B, T, C, H = 8, 1024, 1024, 16
D = C // H
FF = 4 * C
P = 128
NT = T // P      # 8 token tiles
NKC = C // P     # 8 contraction chunks over C
NM = FF // P     # 32 chunks over FF
COND_LEN = 256
TOKEN_LEN = 768
NEG = -1.0e9
BC = 3.0         # exp logit bias (softmax-denominator cancelled)
EPS = 1e-5
BF16 = ml_dtypes.bfloat16
F8 = ml_dtypes.float8_e4m3

_BUILD_CACHE = {}


def _build(flags):
    """Build and compile the per-core Bass program. flags is a tuple of bools:
    (qk_bias, v_bias, p_bias, b1_bias, b2_bias, ln1_aff, ln2_aff)."""
    import concourse.bass as bass
    from concourse import bacc, tile, mybir

    qk_bias, v_bias, p_bias, b1_bias, b2_bias, ln1_aff, ln2_aff = flags
    f32 = mybir.dt.float32
    i32 = mybir.dt.int32
    bf16 = mybir.dt.bfloat16
    fp8 = mybir.dt.float8e4
    AF = mybir.ActivationFunctionType
    OP = mybir.AluOpType
    AX = mybir.AxisListType
    DRM = mybir.MatmulPerfMode.DoubleRow

    nc = bacc.Bacc("TRN2", target_bir_lowering=False, debug=False)

    x_d = nc.dram_tensor("x", [T, C], bf16, kind="ExternalInput")
    wq_d = nc.dram_tensor("wq", [C, C], fp8, kind="ExternalInput")
    wk_d = nc.dram_tensor("wk", [C, C], fp8, kind="ExternalInput")
    wv_d = nc.dram_tensor("wv", [C, C], fp8, kind="ExternalInput")
    wp_d = nc.dram_tensor("wp", [C, C], fp8, kind="ExternalInput")
    w1h_d = nc.dram_tensor("w1h", [P, NM, NKC, P], fp8, kind="ExternalInput")
    w1l_d = nc.dram_tensor("w1l", [P, NM, NKC, P], fp8, kind="ExternalInput")
    w2h_d = nc.dram_tensor("w2h", [P, NKC, NM, P], fp8, kind="ExternalInput")
    w2l_d = nc.dram_tensor("w2l", [P, NKC, NM, P], fp8, kind="ExternalInput")
    cb_d = nc.dram_tensor("cbias", [P, 2], f32, kind="ExternalInput")
    tri_d = nc.dram_tensor("tri", [P, P], bf16, kind="ExternalInput")
    id_d = nc.dram_tensor("ident", [P, P], bf16, kind="ExternalInput")
    out_d = nc.dram_tensor("out", [T, C], f32, kind="ExternalOutput")

    opt_d = {}
    if qk_bias:
        opt_d["bq"] = nc.dram_tensor("bq", [P, NKC], f32, kind="ExternalInput")
        opt_d["bk"] = nc.dram_tensor("bk", [P, NKC], f32, kind="ExternalInput")
    if v_bias:
        opt_d["bv"] = nc.dram_tensor("bv", [1, C], bf16, kind="ExternalInput")
    if p_bias:
        opt_d["bp"] = nc.dram_tensor("bp", [1, C], bf16, kind="ExternalInput")
    if b1_bias:
        opt_d["b1"] = nc.dram_tensor("b1", [P, NM], f32, kind="ExternalInput")
    if b2_bias:
        opt_d["b2"] = nc.dram_tensor("b2", [1, C], bf16, kind="ExternalInput")
    if ln1_aff:
        opt_d["g1"] = nc.dram_tensor("g1", [P, C], f32, kind="ExternalInput")
        opt_d["o1"] = nc.dram_tensor("o1", [P, C], f32, kind="ExternalInput")
    if ln2_aff:
        opt_d["g2"] = nc.dram_tensor("g2", [P, C], f32, kind="ExternalInput")
        opt_d["o2"] = nc.dram_tensor("o2", [P, C], f32, kind="ExternalInput")

    x_re = x_d.ap().rearrange("(t p) c -> p t c", p=P)
    out_re = out_d.ap().rearrange("(t p) c -> p t c", p=P)
    wq_re = wq_d.ap().rearrange("(k p) m -> p k m", p=P)
    wk_re = wk_d.ap().rearrange("(k p) m -> p k m", p=P)
    wv_re = wv_d.ap().rearrange("(k p) m -> p k m", p=P)
    wp_re = wp_d.ap().rearrange("(k p) m -> p k m", p=P)

    with tile.TileContext(nc) as tc:
        import contextlib

        with contextlib.ExitStack() as ctx:
            cpool = ctx.enter_context(tc.tile_pool(name="const", bufs=1))
            xpool = ctx.enter_context(tc.tile_pool(name="xres", bufs=1))
            apool = ctx.enter_context(tc.tile_pool(name="act", bufs=1))
            spool = ctx.enter_context(tc.tile_pool(name="small", bufs=8))
            sqpool = ctx.enter_context(tc.tile_pool(name="sqscr", bufs=2))
            mmps = ctx.enter_context(
                tc.tile_pool(name="mm512", bufs=3, space="PSUM")
            )
            w1p = ctx.enter_context(tc.tile_pool(name="w1p", bufs=3))
            w2p = ctx.enter_context(tc.tile_pool(name="w2p", bufs=3))
            gbpool = ctx.enter_context(tc.tile_pool(name="gbscr", bufs=4))

            tri_sb = cpool.tile([P, P], bf16, tag="tri")
            nc.sync.dma_start(tri_sb[:], tri_d[:])
            id_sb = cpool.tile([P, P], bf16, tag="ident")
            nc.sync.dma_start(id_sb[:], id_d[:])
            cb_sb = cpool.tile([P, 2], f32, tag="cbias")
            nc.sync.dma_start(cb_sb[:], cb_d[:])
            bcn_sb = cpool.tile([P, 1], f32, tag="bcneg")
            nc.vector.memset(bcn_sb[:], -BC)
            magic_sb = cpool.tile([P, 1], i32, tag="magic")
            nc.vector.memset(magic_sb[:], 0x5F3759DF)
            ones_dn = cpool.tile([P, D], fp8, tag="onesdn")
            nc.gpsimd.memset(ones_dn[:], 1.0)
            need_ones_b = v_bias or p_bias or b2_bias
            if need_ones_b:
                ones_b = cpool.tile([1, P], bf16, tag="onesb")
                nc.gpsimd.memset(ones_b[:], 1.0)
            opt_sb = {}
            for nm, dd in opt_d.items():
                shp = list(dd.shape)
                dt_ = dd.dtype
                opt_sb[nm] = cpool.tile(shp, dt_, tag=nm)
                nc.sync.dma_start(opt_sb[nm][:], dd[:])

            x_sb = xpool.tile([P, NT, C], bf16, tag="x")
            for t in range(NT):
                nc.sync.dma_start(x_sb[:, t, :], x_re[:, t, :])

            # ---------------- LayerNorm (token-major) + transpose ----------
            def ln_tile(dst_tok, t, affine, act_mean=False, mean_acc=None,
                        norm_act=False, dve_var=False):
                xr = x_sb[:, t, :]
                mu = spool.tile([P, 1], f32, tag="mu")
                if mean_acc is not None:
                    nc.vector.tensor_add(
                        mu, mean_acc[:, 0:1], mean_acc[:, 1:2]
                    )
                    nc.vector.tensor_scalar_mul(mu, mu, 1.0 / C)
                elif act_mean:
                    cs = sqpool.tile([P, C], bf16, tag="sq")
                    nc.scalar.activation(cs, xr, AF.Copy, accum_out=mu)
                    nc.vector.tensor_scalar_mul(mu, mu, 1.0 / C)
                else:
                    nc.vector.tensor_reduce(mu, xr, axis=AX.X, op=OP.add)
                    nc.vector.tensor_scalar_mul(mu, mu, 1.0 / C)
                sq = sqpool.tile([P, C], bf16, tag="sq")
                ss = spool.tile([P, 1], f32, tag="ss")
                if dve_var:
                    nc.vector.tensor_mul(sq, xr, xr)
                    nc.vector.tensor_reduce(ss, sq, axis=AX.X, op=OP.add)
                else:
                    nc.scalar.activation(sq, xr, AF.Square, accum_out=ss)
                var = spool.tile([P, 1], f32, tag="var")
                musq = spool.tile([P, 1], f32, tag="musq")
                nc.vector.tensor_mul(musq, mu, mu)
                nc.vector.tensor_scalar_mul(var, ss, 1.0 / C)
                nc.vector.tensor_sub(var, var, musq)
                nc.vector.tensor_scalar_add(var, var, EPS)
                rstd = spool.tile([P, 1], f32, tag="rstd")
                ri = rstd[:].bitcast(i32)
                nc.vector.tensor_single_scalar(
                    ri, var[:].bitcast(i32), 1, op=OP.arith_shift_right
                )
                nc.vector.tensor_sub(ri, magic_sb[:], ri)
                nsq = spool.tile([P, 1], f32, tag="nsq")
                for _ in range(2):
                    nc.vector.tensor_mul(nsq, rstd, rstd)
                    nc.vector.tensor_mul(nsq, nsq, var)
                    nc.vector.tensor_scalar(
                        nsq, nsq, -0.5, 1.5, op0=OP.mult, op1=OP.add
                    )
                    nc.vector.tensor_mul(rstd, rstd, nsq)
                if affine is None and norm_act:
                    nmr = spool.tile([P, 1], f32, tag="nmr")
                    nc.vector.tensor_mul(nmr, mu, rstd)
                    nc.vector.tensor_scalar_mul(nmr, nmr, -1.0)
                    nc.scalar.activation(
                        dst_tok[:, t, :], xr, AF.Identity,
                        bias=nmr, scale=rstd,
                    )
                elif affine is None:
                    nc.vector.tensor_scalar(
                        dst_tok[:, t, :], xr, mu, rstd,
                        op0=OP.subtract, op1=OP.mult,
                    )
                else:
                    g_sb_, o_sb_ = affine
                    tmp = spool.tile([P, C], f32, tag="lntmp")
                    nc.vector.tensor_scalar(
                        tmp, xr, mu, rstd, op0=OP.subtract, op1=OP.mult
                    )
                    nc.vector.tensor_mul(tmp, tmp, g_sb_[:])
                    nc.vector.tensor_add(dst_tok[:, t, :], tmp, o_sb_[:])

            def transp_tile(dst8, t, src_tok, psum_pool, lo8=None,
                            act_evict=False):
                """Transpose token tile t of src_tok into feature-major fp8
                dst8 via one packed PSUM bank: 8 transposes [128,128]bf16 into
                one [P,512]f32 tile's bitcast view, then a single packed
                eviction (and optional lo-residual sub)."""
                tp = psum_pool.tile([P, 512], f32, tag="S", name=f"tp{t}")
                tpv = tp[:].bitcast(bf16)          # [P, 1024] bf16 view
                tpr = tpv.rearrange("p (m q) -> p m q", q=P)
                for mc in range(NKC):
                    nc.tensor.transpose(
                        tpr[:, mc, :], src_tok[:, t, mc * P:(mc + 1) * P],
                        id_sb[:],
                    )
                dsl = dst8[:, :, t * P:(t + 1) * P]  # [P, NKC, 128] strided
                if act_evict:
                    nc.scalar.activation(dsl, tpr, AF.Copy)
                else:
                    nc.vector.tensor_copy(dsl, tpr)
                if lo8 is not None:
                    nc.vector.tensor_sub(
                        lo8[:, :, t * P:(t + 1) * P], tpr, dsl
                    )

            ln1_args = (opt_sb["g1"][:], opt_sb["o1"][:]) if ln1_aff else None
            ln2_args = (opt_sb["g2"][:], opt_sb["o2"][:]) if ln2_aff else None

            xn_tok = apool.tile([P, NT, C], bf16, tag="tok")
            xnT8 = apool.tile([P, NKC, T], fp8, tag="fT8")

            # ---------------- QKV + attention + pipelined MLP --------------
            with contextlib.ExitStack() as actx:
                qkvy = actx.enter_context(tc.tile_pool(name="qkvy", bufs=1))
                wpool = actx.enter_context(tc.tile_pool(name="wstream", bufs=3))
                q_sb = qkvy.tile([P, NKC, T], fp8, tag="q")
                k_sb = qkvy.tile([P, NKC, T], fp8, tag="k")
                v_sb = qkvy.tile([P, NT, H, D], fp8, tag="v")
                y_sb = qkvy.tile([P, NKC, T], fp8, tag="y")

                wv_sb = wpool.tile([P, NKC, C], fp8, tag="w")
                nc.sync.dma_start(wv_sb[:], wv_re)
                wq_sb = wpool.tile([P, NKC, C], fp8, tag="w")
                nc.sync.dma_start(wq_sb[:], wq_re)
                wk_sb = wpool.tile([P, NKC, C], fp8, tag="w")
                nc.sync.dma_start(wk_sb[:], wk_re)

                def qk_chunk(w_sb, dst, bias_nm, n2, m):
                    ps = mmps.tile([P, 512], f32, tag="S")
                    for j2 in range(4):
                        nc.tensor.matmul(
                            ps,
                            w_sb[:, 2 * j2:2 * j2 + 2,
                                 m * P:(m + 1) * P],
                            xnT8[:, 2 * j2:2 * j2 + 2,
                                 n2 * 512:(n2 + 1) * 512],
                            start=(j2 == 0),
                            stop=(j2 == 3),
                            perf_mode=DRM,
                        )
                    dsl = dst[:, m, n2 * 512:(n2 + 1) * 512]
                    if qk_bias:
                        nc.scalar.activation(
                            dsl, ps, AF.Identity,
                            bias=opt_sb[bias_nm][:, m:m + 1],
                        )
                    elif m % 2 == 0:
                        nc.scalar.activation(dsl, ps, AF.Copy)
                    else:
                        nc.vector.tensor_copy(dsl, ps)

                def emit_v(t, n2, act_evict=False):
                    ps = mmps.tile([P, 512], f32, tag="S")
                    for j2 in range(4):
                        nc.tensor.matmul(
                            ps,
                            xnT8[:, 2 * j2:2 * j2 + 2, t * P:(t + 1) * P],
                            wv_sb[:, 2 * j2:2 * j2 + 2,
                                  n2 * 512:(n2 + 1) * 512],
                            start=(j2 == 0),
                            stop=(j2 == 3) and not v_bias,
                            perf_mode=DRM,
                        )
                    if v_bias:
                        nc.tensor.matmul(
                            ps, ones_b[:],
                            opt_sb["bv"][:, n2 * 512:(n2 + 1) * 512],
                            start=False, stop=True,
                        )
                    dst = v_sb[:, t, n2 * 8:(n2 + 1) * 8, :]
                    src = ps.rearrange("p (h d) -> p h d", d=D)
                    if act_evict:
                        nc.scalar.activation(dst, src, AF.Copy)
                    else:
                        nc.vector.tensor_copy(dst, src)

                h_tok = apool.tile([P, NT, C], bf16, tag="tok")
                hT_hi = apool.tile([P, NKC, T], fp8, tag="fT8")
                hT_lo = apool.tile([P, NKC, T], fp8, tag="hlo")
                hT_sm = apool.tile([P, NKC, T], fp8, tag="hsm")
                wp_sb = wpool.tile([P, NKC, C], fp8, tag="w")

                def emit_proj(t, n2):
                    ps = mmps.tile([P, 512], f32, tag="S")
                    for j2 in range(4):
                        nc.tensor.matmul(
                            ps,
                            y_sb[:, 2 * j2:2 * j2 + 2, t * P:(t + 1) * P],
                            wp_sb[:, 2 * j2:2 * j2 + 2,
                                  n2 * 512:(n2 + 1) * 512],
                            start=(j2 == 0),
                            stop=(j2 == 3) and not p_bias,
                            perf_mode=DRM,
                        )
                    if p_bias:
                        nc.tensor.matmul(
                            ps, ones_b[:],
                            opt_sb["bp"][:, n2 * 512:(n2 + 1) * 512],
                            start=False, stop=True,
                        )
                    xsl = x_sb[:, t, n2 * 512:(n2 + 1) * 512]
                    if t not in proj_acc:
                        proj_acc[t] = spool.tile([P, 2], f32, tag="pacc",
                                                 name=f"pacc{t}")
                    nc.vector.scalar_tensor_tensor(
                        xsl, ps, 0.0, xsl, op0=OP.add, op1=OP.add,
                        accum_out=proj_acc[t][:, n2:n2 + 1],
                    )

                proj_acc = {}

                # ---- MLP emitters ----
                g_hi = {}
                g_lo = {}
                w1_pre = {}

                def prefetch_w1(m, n2):
                    w1ht = w1p.tile([P, NKC, P], fp8, tag="w1h",
                                    name=f"w1h{m}_{n2}")
                    nc.sync.dma_start(w1ht[:], w1h_d[:, m, :, :])
                    w1lt = w1p.tile([P, NKC, P], fp8, tag="w1l",
                                    name=f"w1l{m}_{n2}")
                    nc.sync.dma_start(w1lt[:], w1l_d[:, m, :, :])
                    w1_pre[(m, n2)] = (w1ht, w1lt)

                def emit_mlp1(m, n2):
                    if n2 not in g_hi:
                        g_hi[n2] = apool.tile([P, NM, 512], fp8, tag="ghi",
                                              name=f"ghi{n2}")
                        g_lo[n2] = apool.tile([P, NM, 512], fp8, tag="glo",
                                              name=f"glo{n2}")
                    if (m, n2) in w1_pre:
                        w1ht, w1lt = w1_pre.pop((m, n2))
                    else:
                        w1ht = w1p.tile([P, NKC, P], fp8, tag="w1h")
                        nc.sync.dma_start(w1ht[:], w1h_d[:, m, :, :])
                        w1lt = w1p.tile([P, NKC, P], fp8, tag="w1l")
                        nc.sync.dma_start(w1lt[:], w1l_d[:, m, :, :])
                    nsl = slice(n2 * 512, (n2 + 1) * 512)
                    ps = mmps.tile([P, 512], f32, tag="S")
                    for xa, wa in ((hT_hi, w1ht), (hT_lo, w1ht), (hT_sm, w1lt)):
                        first = xa is hT_hi
                        for j2 in range(4):
                            nc.tensor.matmul(
                                ps,
                                wa[:, 2 * j2:2 * j2 + 2, :],
                                xa[:, 2 * j2:2 * j2 + 2, nsl],
                                start=(first and j2 == 0),
                                stop=(xa is hT_sm and j2 == 3),
                                perf_mode=DRM,
                            )
                    gsl_h = g_hi[n2][:, m, :]
                    gsl_l = g_lo[n2][:, m, :]
                    gb = gbpool.tile([P, 512], bf16, tag="gb")
                    if b1_bias:
                        nc.scalar.activation(
                            gb, ps, AF.Gelu, bias=opt_sb["b1"][:, m:m + 1])
                    else:
                        nc.scalar.activation(gb, ps, AF.Gelu)
                    if m >= NM - 8:
                        # tail chunks: keep splits off the backlogged Pool
                        # queue so g completes promptly (it gates MLP2)
                        nc.vector.tensor_copy(gsl_h, gb)
                        nc.vector.tensor_sub(gsl_l, gb, gsl_h)
                    elif m % 2 == 0:
                        nc.gpsimd.tensor_copy(gsl_h, gb)
                        nc.vector.tensor_sub(gsl_l, gb, gsl_h)
                    else:
                        nc.vector.tensor_copy(gsl_h, gb)
                        nc.gpsimd.tensor_sub(gsl_l, gb, gsl_h)
                    return gb

                def emit_mlp2_tt(n8, n2, w2ht, w2lt, tt):
                    ghi, glo = g_hi[n2], g_lo[n2]
                    nsl = slice(n8 * P, (n8 + 1) * P)
                    if True:
                        t = n2 * 4 + tt
                        tsl = slice(tt * P, (tt + 1) * P)
                        psAB = mmps.tile([P, 512], f32, tag="S",
                                         name=f"AB{n8}_{tt}")
                        psA = psAB[:, 0:P]
                        psB = psAB[:, P:2 * P]
                        for j2 in range(16):
                            nc.tensor.matmul(
                                psB,
                                ghi[:, 2 * j2:2 * j2 + 2, tsl],
                                w2lt[:, 2 * j2:2 * j2 + 2, :],
                                start=(j2 == 0),
                                stop=(j2 == 15),
                                perf_mode=DRM,
                            )
                        if n2 == 0:
                            # in-sweep half: keep ACT free for exp
                            t1 = gbpool.tile([P, P], f32, tag="t1")
                            nc.vector.scalar_tensor_tensor(
                                t1, psB, float(2.0 ** -5), x_sb[:, t, nsl],
                                op0=OP.mult, op1=OP.add,
                            )
                        for ga in (ghi, glo):
                            first = ga is ghi
                            for j2 in range(16):
                                nc.tensor.matmul(
                                    psA,
                                    ga[:, 2 * j2:2 * j2 + 2, tsl],
                                    w2ht[:, 2 * j2:2 * j2 + 2, :],
                                    start=(first and j2 == 0),
                                    stop=(not first and j2 == 15
                                          and not b2_bias and n2 == 0),
                                    perf_mode=DRM,
                                )
                        if b2_bias:
                            nc.tensor.matmul(
                                psA, ones_b[:], opt_sb["b2"][:, nsl],
                                start=False, stop=(n2 == 0),
                            )
                        if n2 == 1:
                            # tail half: ACT is idle — fold x into psA on
                            # the PE and scale psB on ACT, halving the DVE
                            # eviction cost
                            nc.tensor.matmul(
                                psA, id_sb[:], x_sb[:, t, nsl],
                                start=False, stop=True,
                            )
                            t1 = gbpool.tile([P, P], f32, tag="t1")
                            nc.scalar.activation(
                                t1, psB, AF.Identity,
                                scale=float(2.0 ** -5),
                            )
                        oc = gbpool.tile([P, P], f32, tag="oc")
                        nc.vector.tensor_add(oc, t1, psA)
                        nc.sync.dma_start(out_re[:, t, nsl], oc)

                w2_pre = {}

                def prefetch_w2(n8, n2):
                    w2ht = w2p.tile([P, NM, P], fp8, tag="w2h",
                                    name=f"w2h{n8}_{n2}")
                    nc.sync.dma_start(w2ht[:], w2h_d[:, n8, :, :])
                    w2lt = w2p.tile([P, NM, P], fp8, tag="w2l",
                                    name=f"w2l{n8}_{n2}")
                    nc.sync.dma_start(w2lt[:], w2l_d[:, n8, :, :])
                    w2_pre[(n8, n2)] = (w2ht, w2lt)

                def emit_mlp2_chunk(n8, n2):
                    if (n8, n2) in w2_pre:
                        w2ht, w2lt = w2_pre.pop((n8, n2))
                    else:
                        w2ht = w2p.tile([P, NM, P], fp8, tag="w2h")
                        nc.sync.dma_start(w2ht[:], w2h_d[:, n8, :, :])
                        w2lt = w2p.tile([P, NM, P], fp8, tag="w2l")
                        nc.sync.dma_start(w2lt[:], w2l_d[:, n8, :, :])
                    for tt in range(4):
                        emit_mlp2_tt(n8, n2, w2ht, w2lt, tt)

                def mlp2_pieces(n2):
                    """Generator of tt-granular MLP2 emission thunks for
                    half n2, with rolling w2 prefetch."""
                    for n8 in range(NKC):
                        if (n8, n2) in w2_pre:
                            w2ht, w2lt = w2_pre.pop((n8, n2))
                        else:
                            w2ht = w2p.tile([P, NM, P], fp8, tag="w2h")
                            nc.sync.dma_start(w2ht[:], w2h_d[:, n8, :, :])
                            w2lt = w2p.tile([P, NM, P], fp8, tag="w2l")
                            nc.sync.dma_start(w2lt[:], w2l_d[:, n8, :, :])
                        if n8 + 2 < NKC:
                            prefetch_w2(n8 + 2, n2)
                        for tt in range(4):
                            yield lambda n8=n8, tt=tt, wh=w2ht, wl=w2lt: \
                                emit_mlp2_tt(n8, n2, wh, wl, tt)

                # ---- attention core ----
                with (
                    tc.tile_pool(name="epool", bufs=3) as epool,
                    tc.tile_pool(name="spsum", bufs=2, space="PSUM") as sps,
                    tc.tile_pool(name="ypsum", bufs=2, space="PSUM") as yps,
                    tc.tile_pool(name="dpsum", bufs=1, space="PSUM") as dps,
                    tc.tile_pool(name="attsb", bufs=2) as asb,
                ):
                    e_tiles = {}

                    def kts_for(qc, qq):
                        g = 4 * qc + qq
                        return list(range(2)) + list(range(2, g + 1))

                    def emit_S_cond(h, qc, cbt=None, bcnt=None):
                        """Cond-key S + exp for pair (h, qc): kt0/kt1
                        512 queries wide. cbt/bcnt override the exp bias
                        tiles (used to gate gated-sweep exps behind the
                        last MLP1 gelu)."""
                        if cbt is None:
                            cbt = cb_sb
                        po = (h % 2) * 64
                        mc = h // 2
                        qsl = slice(qc * 512, (qc + 1) * 512)
                        e_c = epool.tile([P, 2, 512], fp8, tag="ec",
                                         name=f"ec{h}_{qc}")
                        e_z = epool.tile([P, 6, 512], fp8, tag="ez",
                                         name=f"ez{h}_{qc}")
                        e_tiles[(h, qc)] = (e_c, e_z)
                        for kt in range(2):
                            s_ps = sps.tile([P, 512], f32, tag="S")
                            nc.tensor.matmul(
                                s_ps,
                                k_sb[po:po + 64, mc, kt * P:(kt + 1) * P]
                                    .unsqueeze(1).broadcast_to([64, 2, P]),
                                q_sb[po:po + 64, mc, qsl]
                                    .unsqueeze(1).broadcast_to([64, 2, 512]),
                                start=True, stop=True,
                                perf_mode=DRM,
                            )
                            nc.scalar.activation(
                                e_c[:, kt, :], s_ps, AF.Exp,
                                bias=cbt[:, kt:kt + 1], scale=0.0625,
                            )
                    def emit_S_causal(h, qc, cbt=None, bcnt=None):
                        if bcnt is None:
                            bcnt = bcn_sb
                        po = (h % 2) * 64
                        mc = h // 2
                        e_c, e_z = e_tiles[(h, qc)]
                        for qq in range(4):
                            g = 4 * qc + qq
                            if g < 2:
                                continue
                            ckts = list(range(2, g + 1))
                            qql = slice(qc * 512 + qq * P,
                                        qc * 512 + (qq + 1) * P)
                            for p0 in range(0, len(ckts), 4):
                                grp = ckts[p0:p0 + 4]
                                s_z = sps.tile([P, 512], f32, tag="S")
                                for idx, kt in enumerate(grp):
                                    zsl = s_z[:, idx * P:(idx + 1) * P]
                                    diag = kt == g
                                    nc.tensor.matmul(
                                        zsl,
                                        k_sb[po:po + 64, mc,
                                             kt * P:(kt + 1) * P]
                                        .unsqueeze(1)
                                        .broadcast_to([64, 2, P]),
                                        q_sb[po:po + 64, mc, qql]
                                        .unsqueeze(1)
                                        .broadcast_to([64, 2, P]),
                                        start=True, stop=not diag,
                                        perf_mode=DRM,
                                    )
                                    if diag:
                                        # causal triangle via PE: I^T @ tri
                                        # accumulates tri into the diag tile
                                        nc.tensor.matmul(
                                            zsl, id_sb[:], tri_sb[:],
                                            start=False, stop=True,
                                        )
                                dst = e_z[:, p0:p0 + len(grp),
                                          qq * P:(qq + 1) * P]
                                nc.scalar.activation(
                                    dst, s_z[:, 0:len(grp) * P], AF.Exp,
                                    bias=bcnt[:], scale=0.0625,
                                )

                    avd_state = {}

                    def emit_AV(h, qc, qqs):
                        """AV + denominator matmuls for pair (h, qc),
                        query tiles qqs only."""
                        if (h, qc) not in avd_state:
                            avd_state[(h, qc)] = (
                                yps.tile([64, 512], f32, tag="Y",
                                         name=f"y{h}_{qc}"),
                                dps.tile([64, 512], f32, tag="DN",
                                         name=f"dn{h}_{qc}"),
                            )
                        y_ps, dn_ps = avd_state[(h, qc)]
                        e_c, e_z = e_tiles[(h, qc)]
                        ones_bc = ones_dn[:].unsqueeze(1) \
                            .broadcast_to([P, 2, D])
                        for qq in qqs:
                            g = 4 * qc + qq
                            kts = kts_for(qc, qq)
                            npair = (len(kts)) // 2
                            single = (len(kts) % 2) == 1
                            ysl = y_ps[:, qq * P:(qq + 1) * P]
                            dsl = dn_ps[:, qq * P:(qq + 1) * P]
                            for kp in range(npair):
                                if kp == 0:
                                    mv = e_c[:, 0:2, qq * P:(qq + 1) * P]
                                else:
                                    mv = e_z[:, 2 * kp - 2:2 * kp,
                                             qq * P:(qq + 1) * P]
                                st = (kp == 0)
                                sp = (kp == npair - 1) and not single
                                nc.tensor.matmul(
                                    ysl, v_sb[:, 2 * kp:2 * kp + 2, h, :],
                                    mv, start=st, stop=sp, perf_mode=DRM,
                                )
                                nc.tensor.matmul(
                                    dsl, ones_bc, mv,
                                    start=st, stop=sp, perf_mode=DRM,
                                )
                            if single:
                                mv = e_z[:, g - 2, qq * P:(qq + 1) * P]
                                nc.tensor.matmul(
                                    ysl, v_sb[:, g, h, :], mv,
                                    start=False, stop=True,
                                )
                                nc.tensor.matmul(
                                    dsl, ones_dn[:], mv,
                                    start=False, stop=True,
                                )
                    def emit_norm(h, qc):
                        po = (h % 2) * 64
                        mc = h // 2
                        e_tiles.pop((h, qc))
                        y_ps, dn_ps = avd_state.pop((h, qc))
                        # normalize: rb = 1/dn  (bf16), y_sb = y_ps * rb
                        rb = asb.tile([64, 512], bf16, tag="rb")
                        with nc.allow_low_precision(
                            reason="uniform per-row softmax scale; bf16 ok"
                        ):
                            nc.vector.reciprocal(rb, dn_ps[:])
                        qsl = slice(qc * 512, (qc + 1) * 512)
                        if po == 0:
                            nc.vector.tensor_mul(
                                y_sb[0:64, mc, qsl], y_ps[:], rb
                            )
                        else:
                            yt = asb.tile([64, 512], fp8, tag="yt")
                            nc.vector.tensor_mul(yt, y_ps[:], rb)
                            nc.sync.dma_start(y_sb[64:128, mc, qsl], yt)

                    def emit_S(h, qc, cbt=None, bcnt=None):
                        emit_S_cond(h, qc, cbt, bcnt)
                        emit_S_causal(h, qc, cbt, bcnt)

                    def emit_AVD(h, qc):
                        emit_AV(h, qc, range(4))
                        emit_norm(h, qc)

                    def emit_ln2_transp(t, act_evict=False):
                        # transposes borrow the S psum ring (attention S is
                        # quiescent whenever LN2 runs)
                        ln_tile(h_tok, t, ln2_args, act_mean=True,
                                mean_acc=proj_acc.pop(t))
                        transp_tile(hT_hi, t, h_tok, sps, lo8=hT_lo,
                                    act_evict=act_evict)
                        sm_dst = hT_sm[:, :, t * P:(t + 1) * P]
                        sm_src = hT_hi[:, :, t * P:(t + 1) * P]
                        if t % 2 == 0:
                            nc.gpsimd.tensor_scalar_mul(
                                sm_dst, sm_src, float(2.0 ** -5))
                        else:
                            nc.vector.tensor_scalar_mul(
                                sm_dst, sm_src, float(2.0 ** -5))

                    # ---- startup: LN1 t0..3, V t0..3, Q/K n2=0
                    # interleaved so the qc0 sweep can start after the
                    # first head's chunks; LN1 t4..7 and V t4..7 become
                    # in-sweep fillers (DVE-side, ACT is exp-bound there) --
                    for t in range(4):
                        ln_tile(xn_tok, t, ln1_args,
                                act_mean=True,
                                norm_act=(t % 2 == 1))
                        transp_tile(xnT8, t, xn_tok, mmps,
                                    act_evict=(t % 2 == 1))
                    for t in range(4):
                        emit_v(t, 0, act_evict=False)
                        emit_v(t, 1, act_evict=True)
                    for m in range(NKC):
                        qk_chunk(wq_sb, q_sb, "bq", 0, m)
                        qk_chunk(wk_sb, k_sb, "bk", 0, m)

                    # ---- phase plan (act tables: exp and gelu can't share
                    # a table, each swap costs 1.28us of ACT — keep phases
                    # table-pure):
                    #   qc0 sweep (exp)  [fill: QK n2=1, wp DMA]
                    #   proj/LN2 t0..3
                    #   MLP1 half-0 (gelu, PE-dense)
                    #   qc1 sweep (exp)  [fill: MLP2 half-0 — no ACT work]
                    #   proj/LN2 t4..7, MLP1 half-1 (gelu), MLP2 half-1

                    fillers = {}

                    def add_filler(i, fn):
                        fillers.setdefault(i, []).append(fn)

                    # --- qc0 sweep ---
                    # odd heads (DMA-shift eviction path) first: the last
                    # norms then write y_sb directly and proj isn't gated
                    # on a trailing DMA-completion semaphore
                    pairs0 = [(h, 0) for h in range(H) if h % 2 == 1] + \
                             [(h, 0) for h in range(H) if h % 2 == 0]

                    def ln1_late(t):
                        ln_tile(xn_tok, t, ln1_args, act_mean=False,
                                norm_act=False, dve_var=(t % 2 == 1))
                        transp_tile(xnT8, t, xn_tok, mmps,
                                    act_evict=(t % 2 == 0))

                    for t in range(4, NT):
                        add_filler(t - 4, lambda t=t: ln1_late(t))
                        add_filler(t - 3, lambda t=t: emit_v(t, 0))
                        add_filler(t - 3, lambda t=t: emit_v(t, 1))
                    for m in range(NKC):
                        add_filler(5 + m, lambda m=m: qk_chunk(
                            wq_sb, q_sb, "bq", 1, m))
                        add_filler(9 + m if m >= 4 else 9 + m // 2 * 2,
                                   lambda m=m: qk_chunk(
                                       wk_sb, k_sb, "bk", 1, m))
                    add_filler(10, lambda: nc.sync.dma_start(wp_sb[:], wp_re))
                    add_filler(14, lambda: prefetch_w1(0, 0))
                    add_filler(15, lambda: prefetch_w1(1, 0))
                    for i in range(len(pairs0) + 2):
                        if i < len(pairs0):
                            emit_S_cond(*pairs0[i])
                        if i >= 2:
                            emit_AV(*pairs0[i - 2], (0, 1))
                        if i < len(pairs0):
                            emit_S_causal(*pairs0[i])
                        if i >= 2:
                            emit_AV(*pairs0[i - 2], (2, 3))
                            emit_norm(*pairs0[i - 2])
                        for fn in fillers.get(i, ()):
                            fn()

                    # --- bubble fill: qc1 heads 0..2 ungated, proj/LN2
                    # t0..3, MLP1 half-0; later qc1 exps are gated behind
                    # the last gelu via bias-tile indirection so the greedy
                    # per-engine scheduler cannot interleave exp and gelu
                    # (each exp<->gelu transition costs a 1.28us act-table
                    # load). ---
                    emit_S(0, 1)
                    emit_S(1, 1)
                    emit_AVD(0, 1)
                    for t in range(4):
                        emit_proj(t, 0)
                        emit_proj(t, 1)
                    emit_S(2, 1)
                    emit_AVD(1, 1)
                    for t in range(2):
                        emit_ln2_transp(t, act_evict=True)
                    emit_S(3, 1)
                    emit_AVD(2, 1)
                    for t in range(2, 4):
                        emit_ln2_transp(t, act_evict=True)
                    emit_AVD(3, 1)
                    gb_last = None
                    for m in range(NM):
                        gb_last = emit_mlp1(m, 0)

                    # gate tiles: copy of the exp biases that depends on the
                    # last gelu output
                    cb1_sb = cpool.tile([P, 2], f32, tag="cb1")
                    nc.vector.scalar_tensor_tensor(
                        cb1_sb, gb_last[:, 0:2], 0.0, cb_sb[:],
                        op0=OP.mult, op1=OP.add,
                    )
                    bcn1_sb = cb1_sb[:, 0:1]  # col 0 = -BC when counts>0
                    bcn1_ok = cpool.tile([P, 1], f32, tag="bcn1")
                    nc.vector.scalar_tensor_tensor(
                        bcn1_ok, gb_last[:, 0:1], 0.0, bcn_sb[:],
                        op0=OP.mult, op1=OP.add,
                    )

                    # --- gated qc1 sweep (exp) with MLP2 half-0 on PE,
                    # interleaved at tt (1.3us) granularity so freshly
                    # emitted S matmuls don't queue behind a 5us chunk ---
                    prefetch_w2(0, 0)
                    prefetch_w2(1, 0)
                    pieces = mlp2_pieces(0)
                    import itertools

                    def take(k):
                        for fn in itertools.islice(pieces, k):
                            fn()

                    pairs1 = [(h, 1) for h in range(4, H) if h % 2 == 1] + \
                             [(h, 1) for h in range(4, H) if h % 2 == 0]
                    consumed = [0]

                    def ctake(k):
                        if consumed[0] < 24:
                            take(k)
                            consumed[0] += k

                    for i in range(len(pairs1) + 2):
                        if i < len(pairs1):
                            emit_S_cond(*pairs1[i], cbt=cb1_sb,
                                        bcnt=bcn1_ok)
                        if i >= 2:
                            emit_AV(*pairs1[i - 2], (0, 1))
                        ctake(1)
                        if i < len(pairs1):
                            emit_S_causal(*pairs1[i], cbt=cb1_sb,
                                          bcnt=bcn1_ok)
                        ctake(1)
                        if i >= 2:
                            emit_AV(*pairs1[i - 2], (2, 3))
                            emit_norm(*pairs1[i - 2])

                    # ---- tail: proj/LN2 t4..7, MLP1 h1 (gelu), MLP2 h1 ----
                    prefetch_w1(0, 1)
                    prefetch_w1(1, 1)
                    for t in range(4, NT):
                        emit_proj(t, 0)
                        emit_proj(t, 1)
                        take(2)
                        emit_ln2_transp(t, act_evict=True)
                    take(100)
                    for m in range(NM):
                        emit_mlp1(m, 1)
                        if m == 24:
                            prefetch_w2(0, 1)
                    for n8 in range(NKC):
                        emit_mlp2_chunk(n8, 1)
                # end attention scope

    nc.compile()
    return nc


def _host_aux(cond_mask):
    counts = np.asarray(cond_mask).sum(axis=-1).astype(np.int64)  # [B]
    cbias = []
    for b in range(B):
        vec = np.full(COND_LEN, -BC, np.float32)
        vec[counts[b]:] = NEG
        cbias.append(np.ascontiguousarray(vec.reshape(2, P).T))
    kk = np.arange(P)[:, None]
    qq = np.arange(P)[None, :]
    tri = np.where(qq >= kk, 0.0, NEG).astype(BF16)
    ident = np.eye(P, dtype=BF16)
    return cbias, tri, ident


def kernel(**inputs):
    from concourse.bass_utils import run_bass_kernel_spmd

    x = np.asarray(inputs["x"], np.float32)
    assert x.shape == (B, T, C)
    assert int(inputs["cond_len"]) == COND_LEN
    assert int(inputs["token_len"]) == TOKEN_LEN

    f32 = np.float32
    Wq, Wk, Wv, Wp = (np.asarray(inputs[k], f32) for k in ("Wq", "Wk", "Wv", "Wp"))
    W1, W2 = np.asarray(inputs["W1"], f32), np.asarray(inputs["W2"], f32)
    bq, bk, bv, bp = (np.asarray(inputs[k], f32) for k in ("bq", "bk", "bv", "bp"))
    b1, b2 = np.asarray(inputs["b1"], f32), np.asarray(inputs["b2"], f32)
    g1, o1 = np.asarray(inputs["ln1_g"], f32), np.asarray(inputs["ln1_b"], f32)
    g2, o2 = np.asarray(inputs["ln2_g"], f32), np.asarray(inputs["ln2_b"], f32)

    flags = (
        bool(bq.any() or bk.any()),
        bool(bv.any()),
        bool(bp.any()),
        bool(b1.any()),
        bool(b2.any()),
        bool((g1 != 1).any() or o1.any()),
        bool((g2 != 1).any() or o2.any()),
    )
    if flags not in _BUILD_CACHE:
        _BUILD_CACHE[flags] = _build(flags)
    nc = _BUILD_CACHE[flags]
    qk_bias, v_bias, p_bias, b1_bias, b2_bias, ln1_aff, ln2_aff = flags

    cbias, tri, ident = _host_aux(inputs["cond_mask"])
    w1h = W1.astype(F8)
    w1l = ((W1 - w1h.astype(f32)) * 32.0).astype(F8)
    w2h = W2.astype(F8)
    w2l = ((W2 - w2h.astype(f32)) * 32.0).astype(F8)

    def lay1(w):  # [C, FF] -> [P, NM, NKC, P] chunk-contiguous
        return np.ascontiguousarray(
            w.reshape(NKC, P, NM, P).transpose(1, 2, 0, 3))

    def lay2(w):  # [FF, C] -> [P, NKC, NM, P]
        return np.ascontiguousarray(
            w.reshape(NM, P, NKC, P).transpose(1, 2, 0, 3))

    shared = {
        "wq": Wq.astype(F8), "wk": Wk.astype(F8),
        "wv": Wv.astype(F8), "wp": Wp.astype(F8),
        "w1h": lay1(w1h), "w1l": lay1(w1l),
        "w2h": lay2(w2h), "w2l": lay2(w2l),
        "tri": tri, "ident": ident,
    }
    if qk_bias:
        shared["bq"] = np.ascontiguousarray(bq.reshape(NKC, P).T)
        shared["bk"] = np.ascontiguousarray(bk.reshape(NKC, P).T)
    if v_bias:
        shared["bv"] = bv.reshape(1, C).astype(BF16)
    if p_bias:
        shared["bp"] = bp.reshape(1, C).astype(BF16)
    if b1_bias:
        shared["b1"] = np.ascontiguousarray(b1.reshape(NM, P).T)
    if b2_bias:
        shared["b2"] = b2.reshape(1, C).astype(BF16)
    if ln1_aff:
        shared["g1"] = np.broadcast_to(g1, (P, C)).copy()
        shared["o1"] = np.broadcast_to(o1, (P, C)).copy()
    if ln2_aff:
        shared["g2"] = np.broadcast_to(g2, (P, C)).copy()
        shared["o2"] = np.broadcast_to(o2, (P, C)).copy()

    in_maps = [dict(shared, x=x[b].astype(BF16), cbias=cbias[b])
               for b in range(B)]
    try:
        res = run_bass_kernel_spmd(nc, in_maps, list(range(B)),
                                   trace=kernel._trace)
    except ModuleNotFoundError:
        res = run_bass_kernel_spmd(nc, in_maps, list(range(B)), trace=False)
    kernel._last_results = res
    out = np.stack([res.results[b]["out"] for b in range(B)], axis=0)
    return out.astype(np.float32)


kernel._trace = False
kernel._last_results = None


# revision 12
# speedup vs baseline: 1.0692x; 1.0043x over previous
"""Trainium2 Bass kernel for nn_Block_84155589198355 (dense transformer block).

Data-parallel B=8 over 8 cores; fp8 DoubleRow matmuls everywhere.

v2 changes vs baseline:
  - qt-granular causal attention: S/exp computed only on visible (qt, kt)
    tiles; no masked-region exp, no e-zeroing memsets, no tri on dead tiles.
  - V stored unpadded [128, t, h, 64]; AV emits per-head y_ps [64, 512]
    (DR dst must be partition-base 0). Softmax denominators come from
    dedicated ones-stationary DR matmuls with 64-row replicated output
    (PE cost = moving width only), so the reciprocal+normalize is one
    [64,512] DVE pair per (head, qc) instead of [1,512] ops + partition
    broadcasts. Odd heads (po=64) are DMA-shifted into y_sb[64:128].
  - MLP1 half-0 is interleaved into the qc1 attention sweep: the sweep is
    ACT-bound (exp) while PE idles; act-table swaps are free here.
  - exp of causal tiles is packed: one ACT instruction covers up to 4
    kt-tiles of a (h, qt) group.

Layouts (per core, T=1024, C=1024, H=16, D=64):
  - Q/K feature-major [128, NKC, T] fp8, head pair per chunk (po=(h%2)*64).
  - S cond (kt 0,1): [128k, 512q] stride-0-broadcast DR (2*K^T Q, exp scale
    0.0625); causal: [128k, 128q] per (h, qt, kt), diag tile gets tri add.
  - V token-major [128, t, h, 64] fp8 unpadded.
  - MLP1/MLP2 identical to baseline (3-term hi/lo fp8 splits).
"""

import sys

if "/opt/trn_rl_repo" not in sys.path:
    sys.path.insert(0, "/opt/trn_rl_repo")

import numpy as np
import ml_dtypes

B, T, C, H = 8, 1024, 1024, 16
D = C // H
FF = 4 * C
P = 128
NT = T // P      # 8 token tiles
NKC = C // P     # 8 contraction chunks over C
NM = FF // P     # 32 chunks over FF
COND_LEN = 256
TOKEN_LEN = 768
NEG = -1.0e9
BC = 3.0         # exp logit bias (softmax-denominator cancelled)
EPS = 1e-5
BF16 = ml_dtypes.bfloat16
F8 = ml_dtypes.float8_e4m3

_BUILD_CACHE = {}


def _build(flags):
    """Build and compile the per-core Bass program. flags is a tuple of bools:
    (qk_bias, v_bias, p_bias, b1_bias, b2_bias, ln1_aff, ln2_aff)."""
    import concourse.bass as bass
    from concourse import bacc, tile, mybir

    qk_bias, v_bias, p_bias, b1_bias, b2_bias, ln1_aff, ln2_aff = flags
    f32 = mybir.dt.float32
    i32 = mybir.dt.int32
    bf16 = mybir.dt.bfloat16
    fp8 = mybir.dt.float8e4
    AF = mybir.ActivationFunctionType
    OP = mybir.AluOpType
    AX = mybir.AxisListType
    DRM = mybir.MatmulPerfMode.DoubleRow

    nc = bacc.Bacc("TRN2", target_bir_lowering=False, debug=False)

    x_d = nc.dram_tensor("x", [T, C], bf16, kind="ExternalInput")
    wq_d = nc.dram_tensor("wq", [C, C], fp8, kind="ExternalInput")
    wk_d = nc.dram_tensor("wk", [C, C], fp8, kind="ExternalInput")
    wv_d = nc.dram_tensor("wv", [C, C], fp8, kind="ExternalInput")
    wp_d = nc.dram_tensor("wp", [C, C], fp8, kind="ExternalInput")
    w1h_d = nc.dram_tensor("w1h", [P, NM, NKC, P], fp8, kind="ExternalInput")
    w1l_d = nc.dram_tensor("w1l", [P, NM, NKC, P], fp8, kind="ExternalInput")
    w2h_d = nc.dram_tensor("w2h", [P, NKC, NM, P], fp8, kind="ExternalInput")
    w2l_d = nc.dram_tensor("w2l", [P, NKC, NM, P], fp8, kind="ExternalInput")
    cb_d = nc.dram_tensor("cbias", [P, 2], f32, kind="ExternalInput")
    tri_d = nc.dram_tensor("tri", [P, P], bf16, kind="ExternalInput")
    id_d = nc.dram_tensor("ident", [P, P], bf16, kind="ExternalInput")
    out_d = nc.dram_tensor("out", [T, C], f32, kind="ExternalOutput")

    opt_d = {}
    if qk_bias:
        opt_d["bq"] = nc.dram_tensor("bq", [P, NKC], f32, kind="ExternalInput")
        opt_d["bk"] = nc.dram_tensor("bk", [P, NKC], f32, kind="ExternalInput")
    if v_bias:
        opt_d["bv"] = nc.dram_tensor("bv", [1, C], bf16, kind="ExternalInput")
    if p_bias:
        opt_d["bp"] = nc.dram_tensor("bp", [1, C], bf16, kind="ExternalInput")
    if b1_bias:
        opt_d["b1"] = nc.dram_tensor("b1", [P, NM], f32, kind="ExternalInput")
    if b2_bias:
        opt_d["b2"] = nc.dram_tensor("b2", [1, C], bf16, kind="ExternalInput")
    if ln1_aff:
        opt_d["g1"] = nc.dram_tensor("g1", [P, C], f32, kind="ExternalInput")
        opt_d["o1"] = nc.dram_tensor("o1", [P, C], f32, kind="ExternalInput")
    if ln2_aff:
        opt_d["g2"] = nc.dram_tensor("g2", [P, C], f32, kind="ExternalInput")
        opt_d["o2"] = nc.dram_tensor("o2", [P, C], f32, kind="ExternalInput")

    x_re = x_d.ap().rearrange("(t p) c -> p t c", p=P)
    out_re = out_d.ap().rearrange("(t p) c -> p t c", p=P)
    wq_re = wq_d.ap().rearrange("(k p) m -> p k m", p=P)
    wk_re = wk_d.ap().rearrange("(k p) m -> p k m", p=P)
    wv_re = wv_d.ap().rearrange("(k p) m -> p k m", p=P)
    wp_re = wp_d.ap().rearrange("(k p) m -> p k m", p=P)

    with tile.TileContext(nc) as tc:
        import contextlib

        with contextlib.ExitStack() as ctx:
            cpool = ctx.enter_context(tc.tile_pool(name="const", bufs=1))
            xpool = ctx.enter_context(tc.tile_pool(name="xres", bufs=1))
            apool = ctx.enter_context(tc.tile_pool(name="act", bufs=1))
            spool = ctx.enter_context(tc.tile_pool(name="small", bufs=8))
            sqpool = ctx.enter_context(tc.tile_pool(name="sqscr", bufs=3))
            mmps = ctx.enter_context(
                tc.tile_pool(name="mm512", bufs=3, space="PSUM")
            )
            w1p = ctx.enter_context(tc.tile_pool(name="w1p", bufs=3))
            w2p = ctx.enter_context(tc.tile_pool(name="w2p", bufs=3))
            gbpool = ctx.enter_context(tc.tile_pool(name="gbscr", bufs=4))

            tri_sb = cpool.tile([P, P], bf16, tag="tri")
            nc.sync.dma_start(tri_sb[:], tri_d[:])
            id_sb = cpool.tile([P, P], bf16, tag="ident")
            nc.sync.dma_start(id_sb[:], id_d[:])
            cb_sb = cpool.tile([P, 2], f32, tag="cbias")
            nc.sync.dma_start(cb_sb[:], cb_d[:])
            bcn_sb = cpool.tile([P, 1], f32, tag="bcneg")
            nc.vector.memset(bcn_sb[:], -BC)
            magic_sb = cpool.tile([P, 1], i32, tag="magic")
            nc.vector.memset(magic_sb[:], 0x5F3759DF)
            ones_dn = cpool.tile([P, D], fp8, tag="onesdn")
            nc.gpsimd.memset(ones_dn[:], 1.0)
            need_ones_b = v_bias or p_bias or b2_bias
            if need_ones_b:
                ones_b = cpool.tile([1, P], bf16, tag="onesb")
                nc.gpsimd.memset(ones_b[:], 1.0)
            opt_sb = {}
            for nm, dd in opt_d.items():
                shp = list(dd.shape)
                dt_ = dd.dtype
                opt_sb[nm] = cpool.tile(shp, dt_, tag=nm)
                nc.sync.dma_start(opt_sb[nm][:], dd[:])

            x_sb = xpool.tile([P, NT, C], bf16, tag="x")
            for t in range(NT):
                nc.sync.dma_start(x_sb[:, t, :], x_re[:, t, :])

            # ---------------- LayerNorm (token-major) + transpose ----------
            def ln_tile(dst_tok, t, affine, act_mean=False, mean_acc=None,
                        norm_act=False, dve_var=False):
                xr = x_sb[:, t, :]
                mu = spool.tile([P, 1], f32, tag="mu")
                if mean_acc is not None:
                    nc.vector.tensor_add(
                        mu, mean_acc[:, 0:1], mean_acc[:, 1:2]
                    )
                    nc.vector.tensor_scalar_mul(mu, mu, 1.0 / C)
                elif act_mean:
                    cs = sqpool.tile([P, C], bf16, tag="sq")
                    nc.scalar.activation(cs, xr, AF.Copy, accum_out=mu)
                    nc.vector.tensor_scalar_mul(mu, mu, 1.0 / C)
                else:
                    nc.vector.tensor_reduce(mu, xr, axis=AX.X, op=OP.add)
                    nc.vector.tensor_scalar_mul(mu, mu, 1.0 / C)
                sq = sqpool.tile([P, C], bf16, tag="sq")
                ss = spool.tile([P, 1], f32, tag="ss")
                if dve_var:
                    nc.vector.tensor_mul(sq, xr, xr)
                    nc.vector.tensor_reduce(ss, sq, axis=AX.X, op=OP.add)
                else:
                    nc.scalar.activation(sq, xr, AF.Square, accum_out=ss)
                var = spool.tile([P, 1], f32, tag="var")
                musq = spool.tile([P, 1], f32, tag="musq")
                nc.vector.tensor_mul(musq, mu, mu)
                nc.vector.tensor_scalar_mul(var, ss, 1.0 / C)
                nc.vector.tensor_sub(var, var, musq)
                nc.vector.tensor_scalar_add(var, var, EPS)
                rstd = spool.tile([P, 1], f32, tag="rstd")
                ri = rstd[:].bitcast(i32)
                nc.vector.tensor_single_scalar(
                    ri, var[:].bitcast(i32), 1, op=OP.arith_shift_right
                )
                nc.vector.tensor_sub(ri, magic_sb[:], ri)
                nsq = spool.tile([P, 1], f32, tag="nsq")
                for _ in range(2):
                    nc.vector.tensor_mul(nsq, rstd, rstd)
                    nc.vector.tensor_mul(nsq, nsq, var)
                    nc.vector.tensor_scalar(
                        nsq, nsq, -0.5, 1.5, op0=OP.mult, op1=OP.add
                    )
                    nc.vector.tensor_mul(rstd, rstd, nsq)
                if affine is None and norm_act:
                    nmr = spool.tile([P, 1], f32, tag="nmr")
                    nc.vector.tensor_mul(nmr, mu, rstd)
                    nc.vector.tensor_scalar_mul(nmr, nmr, -1.0)
                    nc.scalar.activation(
                        dst_tok[:, t, :], xr, AF.Identity,
                        bias=nmr, scale=rstd,
                    )
                elif affine is None:
                    nc.vector.tensor_scalar(
                        dst_tok[:, t, :], xr, mu, rstd,
                        op0=OP.subtract, op1=OP.mult,
                    )
                else:
                    g_sb_, o_sb_ = affine
                    tmp = spool.tile([P, C], f32, tag="lntmp")
                    nc.vector.tensor_scalar(
                        tmp, xr, mu, rstd, op0=OP.subtract, op1=OP.mult
                    )
                    nc.vector.tensor_mul(tmp, tmp, g_sb_[:])
                    nc.vector.tensor_add(dst_tok[:, t, :], tmp, o_sb_[:])

            def transp_tile(dst8, t, src_tok, psum_pool, lo8=None,
                            act_evict=False):
                """Transpose token tile t of src_tok into feature-major fp8
                dst8 via one packed PSUM bank: 8 transposes [128,128]bf16 into
                one [P,512]f32 tile's bitcast view, then a single packed
                eviction (and optional lo-residual sub)."""
                tp = psum_pool.tile([P, 512], f32, tag="S", name=f"tp{t}")
                tpv = tp[:].bitcast(bf16)          # [P, 1024] bf16 view
                tpr = tpv.rearrange("p (m q) -> p m q", q=P)
                for mc in range(NKC):
                    nc.tensor.transpose(
                        tpr[:, mc, :], src_tok[:, t, mc * P:(mc + 1) * P],
                        id_sb[:],
                    )
                dsl = dst8[:, :, t * P:(t + 1) * P]  # [P, NKC, 128] strided
                if act_evict:
                    nc.scalar.activation(dsl, tpr, AF.Copy)
                else:
                    nc.vector.tensor_copy(dsl, tpr)
                if lo8 is not None:
                    nc.vector.tensor_sub(
                        lo8[:, :, t * P:(t + 1) * P], tpr, dsl
                    )

            ln1_args = (opt_sb["g1"][:], opt_sb["o1"][:]) if ln1_aff else None
            ln2_args = (opt_sb["g2"][:], opt_sb["o2"][:]) if ln2_aff else None

            xn_tok = apool.tile([P, NT, C], bf16, tag="tok")
            xnT8 = apool.tile([P, NKC, T], fp8, tag="fT8")

            # ---------------- QKV + attention + pipelined MLP --------------
            with contextlib.ExitStack() as actx:
                qkvy = actx.enter_context(tc.tile_pool(name="qkvy", bufs=1))
                wpool = actx.enter_context(tc.tile_pool(name="wstream", bufs=3))
                q_sb = qkvy.tile([P, NKC, T], fp8, tag="q")
                k_sb = qkvy.tile([P, NKC, T], fp8, tag="k")
                v_sb = qkvy.tile([P, NT, H, D], fp8, tag="v")
                y_sb = qkvy.tile([P, NKC, T], fp8, tag="y")

                wv_sb = wpool.tile([P, NKC, C], fp8, tag="w")
                nc.sync.dma_start(wv_sb[:], wv_re)
                wq_sb = wpool.tile([P, NKC, C], fp8, tag="w")
                nc.sync.dma_start(wq_sb[:], wq_re)
                wk_sb = wpool.tile([P, NKC, C], fp8, tag="w")
                nc.sync.dma_start(wk_sb[:], wk_re)

                def qk_chunk(w_sb, dst, bias_nm, n2, m):
                    ps = mmps.tile([P, 512], f32, tag="S")
                    for j2 in range(4):
                        nc.tensor.matmul(
                            ps,
                            w_sb[:, 2 * j2:2 * j2 + 2,
                                 m * P:(m + 1) * P],
                            xnT8[:, 2 * j2:2 * j2 + 2,
                                 n2 * 512:(n2 + 1) * 512],
                            start=(j2 == 0),
                            stop=(j2 == 3),
                            perf_mode=DRM,
                        )
                    dsl = dst[:, m, n2 * 512:(n2 + 1) * 512]
                    if qk_bias:
                        nc.scalar.activation(
                            dsl, ps, AF.Identity,
                            bias=opt_sb[bias_nm][:, m:m + 1],
                        )
                    elif m % 2 == 0:
                        nc.scalar.activation(dsl, ps, AF.Copy)
                    else:
                        nc.vector.tensor_copy(dsl, ps)

                def emit_v(t, n2, act_evict=False):
                    ps = mmps.tile([P, 512], f32, tag="S")
                    for j2 in range(4):
                        nc.tensor.matmul(
                            ps,
                            xnT8[:, 2 * j2:2 * j2 + 2, t * P:(t + 1) * P],
                            wv_sb[:, 2 * j2:2 * j2 + 2,
                                  n2 * 512:(n2 + 1) * 512],
                            start=(j2 == 0),
                            stop=(j2 == 3) and not v_bias,
                            perf_mode=DRM,
                        )
                    if v_bias:
                        nc.tensor.matmul(
                            ps, ones_b[:],
                            opt_sb["bv"][:, n2 * 512:(n2 + 1) * 512],
                            start=False, stop=True,
                        )
                    dst = v_sb[:, t, n2 * 8:(n2 + 1) * 8, :]
                    src = ps.rearrange("p (h d) -> p h d", d=D)
                    if act_evict:
                        nc.scalar.activation(dst, src, AF.Copy)
                    else:
                        nc.vector.tensor_copy(dst, src)

                h_tok = apool.tile([P, NT, C], bf16, tag="tok")
                hT_hi = apool.tile([P, NKC, T], fp8, tag="fT8")
                hT_lo = apool.tile([P, NKC, T], fp8, tag="hlo")
                hT_sm = apool.tile([P, NKC, T], fp8, tag="hsm")
                wp_sb = wpool.tile([P, NKC, C], fp8, tag="w")

                def emit_proj(t, n2):
                    ps = mmps.tile([P, 512], f32, tag="S")
                    for j2 in range(4):
                        nc.tensor.matmul(
                            ps,
                            y_sb[:, 2 * j2:2 * j2 + 2, t * P:(t + 1) * P],
                            wp_sb[:, 2 * j2:2 * j2 + 2,
                                  n2 * 512:(n2 + 1) * 512],
                            start=(j2 == 0),
                            stop=(j2 == 3) and not p_bias,
                            perf_mode=DRM,
                        )
                    if p_bias:
                        nc.tensor.matmul(
                            ps, ones_b[:],
                            opt_sb["bp"][:, n2 * 512:(n2 + 1) * 512],
                            start=False, stop=True,
                        )
                    xsl = x_sb[:, t, n2 * 512:(n2 + 1) * 512]
                    if t not in proj_acc:
                        proj_acc[t] = spool.tile([P, 2], f32, tag="pacc",
                                                 name=f"pacc{t}")
                    nc.vector.scalar_tensor_tensor(
                        xsl, ps, 0.0, xsl, op0=OP.add, op1=OP.add,
                        accum_out=proj_acc[t][:, n2:n2 + 1],
                    )

                proj_acc = {}

                # ---- MLP emitters ----
                g_hi = {}
                g_lo = {}
                w1_pre = {}

                def prefetch_w1(m, n2):
                    w1ht = w1p.tile([P, NKC, P], fp8, tag="w1h",
                                    name=f"w1h{m}_{n2}")
                    nc.sync.dma_start(w1ht[:], w1h_d[:, m, :, :])
                    w1lt = w1p.tile([P, NKC, P], fp8, tag="w1l",
                                    name=f"w1l{m}_{n2}")
                    nc.sync.dma_start(w1lt[:], w1l_d[:, m, :, :])
                    w1_pre[(m, n2)] = (w1ht, w1lt)

                def emit_mlp1(m, n2):
                    if n2 not in g_hi:
                        g_hi[n2] = apool.tile([P, NM, 512], fp8, tag="ghi",
                                              name=f"ghi{n2}")
                        g_lo[n2] = apool.tile([P, NM, 512], fp8, tag="glo",
                                              name=f"glo{n2}")
                    if (m, n2) in w1_pre:
                        w1ht, w1lt = w1_pre.pop((m, n2))
                    else:
                        w1ht = w1p.tile([P, NKC, P], fp8, tag="w1h")
                        nc.sync.dma_start(w1ht[:], w1h_d[:, m, :, :])
                        w1lt = w1p.tile([P, NKC, P], fp8, tag="w1l")
                        nc.sync.dma_start(w1lt[:], w1l_d[:, m, :, :])
                    nsl = slice(n2 * 512, (n2 + 1) * 512)
                    ps = mmps.tile([P, 512], f32, tag="S")
                    for xa, wa in ((hT_hi, w1ht), (hT_lo, w1ht), (hT_sm, w1lt)):
                        first = xa is hT_hi
                        for j2 in range(4):
                            nc.tensor.matmul(
                                ps,
                                wa[:, 2 * j2:2 * j2 + 2, :],
                                xa[:, 2 * j2:2 * j2 + 2, nsl],
                                start=(first and j2 == 0),
                                stop=(xa is hT_sm and j2 == 3),
                                perf_mode=DRM,
                            )
                    gsl_h = g_hi[n2][:, m, :]
                    gsl_l = g_lo[n2][:, m, :]
                    gb = gbpool.tile([P, 512], bf16, tag="gb")
                    if b1_bias:
                        nc.scalar.activation(
                            gb, ps, AF.Gelu, bias=opt_sb["b1"][:, m:m + 1])
                    else:
                        nc.scalar.activation(gb, ps, AF.Gelu)
                    if m >= NM - 8:
                        # tail chunks: keep splits off the backlogged Pool
                        # queue so g completes promptly (it gates MLP2)
                        nc.vector.tensor_copy(gsl_h, gb)
                        nc.vector.tensor_sub(gsl_l, gb, gsl_h)
                    elif m % 2 == 0:
                        nc.gpsimd.tensor_copy(gsl_h, gb)
                        nc.vector.tensor_sub(gsl_l, gb, gsl_h)
                    else:
                        nc.vector.tensor_copy(gsl_h, gb)
                        nc.gpsimd.tensor_sub(gsl_l, gb, gsl_h)
                    return gb

                def emit_mlp2_tt(n8, n2, w2ht, w2lt, tt):
                    ghi, glo = g_hi[n2], g_lo[n2]
                    nsl = slice(n8 * P, (n8 + 1) * P)
                    if True:
                        t = n2 * 4 + tt
                        tsl = slice(tt * P, (tt + 1) * P)
                        psAB = mmps.tile([P, 512], f32, tag="S",
                                         name=f"AB{n8}_{tt}")
                        psA = psAB[:, 0:P]
                        psB = psAB[:, P:2 * P]
                        for j2 in range(16):
                            nc.tensor.matmul(
                                psB,
                                ghi[:, 2 * j2:2 * j2 + 2, tsl],
                                w2lt[:, 2 * j2:2 * j2 + 2, :],
                                start=(j2 == 0),
                                stop=(j2 == 15),
                                perf_mode=DRM,
                            )
                        if n2 == 0:
                            # in-sweep half: keep ACT free for exp
                            t1 = gbpool.tile([P, P], f32, tag="t1")
                            nc.vector.scalar_tensor_tensor(
                                t1, psB, float(2.0 ** -5), x_sb[:, t, nsl],
                                op0=OP.mult, op1=OP.add,
                            )
                        for ga in (ghi, glo):
                            first = ga is ghi
                            for j2 in range(16):
                                nc.tensor.matmul(
                                    psA,
                                    ga[:, 2 * j2:2 * j2 + 2, tsl],
                                    w2ht[:, 2 * j2:2 * j2 + 2, :],
                                    start=(first and j2 == 0),
                                    stop=(not first and j2 == 15
                                          and not b2_bias and n2 == 0),
                                    perf_mode=DRM,
                                )
                        if b2_bias:
                            nc.tensor.matmul(
                                psA, ones_b[:], opt_sb["b2"][:, nsl],
                                start=False, stop=(n2 == 0),
                            )
                        if n2 == 1:
                            # tail half: ACT is idle — fold x into psA on
                            # the PE and scale psB on ACT, halving the DVE
                            # eviction cost
                            nc.tensor.matmul(
                                psA, id_sb[:], x_sb[:, t, nsl],
                                start=False, stop=True,
                            )
                            t1 = gbpool.tile([P, P], f32, tag="t1")
                            nc.scalar.activation(
                                t1, psB, AF.Identity,
                                scale=float(2.0 ** -5),
                            )
                        oc = gbpool.tile([P, P], f32, tag="oc")
                        nc.vector.tensor_add(oc, t1, psA)
                        nc.sync.dma_start(out_re[:, t, nsl], oc)

                w2_pre = {}

                def prefetch_w2(n8, n2):
                    w2ht = w2p.tile([P, NM, P], fp8, tag="w2h",
                                    name=f"w2h{n8}_{n2}")
                    nc.sync.dma_start(w2ht[:], w2h_d[:, n8, :, :])
                    w2lt = w2p.tile([P, NM, P], fp8, tag="w2l",
                                    name=f"w2l{n8}_{n2}")
                    nc.sync.dma_start(w2lt[:], w2l_d[:, n8, :, :])
                    w2_pre[(n8, n2)] = (w2ht, w2lt)

                def emit_mlp2_chunk(n8, n2):
                    if (n8, n2) in w2_pre:
                        w2ht, w2lt = w2_pre.pop((n8, n2))
                    else:
                        w2ht = w2p.tile([P, NM, P], fp8, tag="w2h")
                        nc.sync.dma_start(w2ht[:], w2h_d[:, n8, :, :])
                        w2lt = w2p.tile([P, NM, P], fp8, tag="w2l")
                        nc.sync.dma_start(w2lt[:], w2l_d[:, n8, :, :])
                    for tt in range(4):
                        emit_mlp2_tt(n8, n2, w2ht, w2lt, tt)

                def mlp2_pieces(n2):
                    """Generator of tt-granular MLP2 emission thunks for
                    half n2, with rolling w2 prefetch."""
                    for n8 in range(NKC):
                        if (n8, n2) in w2_pre:
                            w2ht, w2lt = w2_pre.pop((n8, n2))
                        else:
                            w2ht = w2p.tile([P, NM, P], fp8, tag="w2h")
                            nc.sync.dma_start(w2ht[:], w2h_d[:, n8, :, :])
                            w2lt = w2p.tile([P, NM, P], fp8, tag="w2l")
                            nc.sync.dma_start(w2lt[:], w2l_d[:, n8, :, :])
                        if n8 + 2 < NKC:
                            prefetch_w2(n8 + 2, n2)
                        for tt in range(4):
                            yield lambda n8=n8, tt=tt, wh=w2ht, wl=w2lt: \
                                emit_mlp2_tt(n8, n2, wh, wl, tt)

                # ---- attention core ----
                with (
                    tc.tile_pool(name="epool", bufs=3) as epool,
                    tc.tile_pool(name="spsum", bufs=2, space="PSUM") as sps,
                    tc.tile_pool(name="ypsum", bufs=2, space="PSUM") as yps,
                    tc.tile_pool(name="dpsum", bufs=1, space="PSUM") as dps,
                    tc.tile_pool(name="attsb", bufs=2) as asb,
                ):
                    e_tiles = {}

                    def kts_for(qc, qq):
                        g = 4 * qc + qq
                        return list(range(2)) + list(range(2, g + 1))

                    def emit_S_cond(h, qc, cbt=None, bcnt=None):
                        """Cond-key S + exp for pair (h, qc): kt0/kt1
                        512 queries wide. cbt/bcnt override the exp bias
                        tiles (used to gate gated-sweep exps behind the
                        last MLP1 gelu)."""
                        if cbt is None:
                            cbt = cb_sb
                        po = (h % 2) * 64
                        mc = h // 2
                        qsl = slice(qc * 512, (qc + 1) * 512)
                        e_c = epool.tile([P, 2, 512], fp8, tag="ec",
                                         name=f"ec{h}_{qc}")
                        e_z = epool.tile([P, 6, 512], fp8, tag="ez",
                                         name=f"ez{h}_{qc}")
                        e_tiles[(h, qc)] = (e_c, e_z)
                        for kt in range(2):
                            s_ps = sps.tile([P, 512], f32, tag="S")
                            nc.tensor.matmul(
                                s_ps,
                                k_sb[po:po + 64, mc, kt * P:(kt + 1) * P]
                                    .unsqueeze(1).broadcast_to([64, 2, P]),
                                q_sb[po:po + 64, mc, qsl]
                                    .unsqueeze(1).broadcast_to([64, 2, 512]),
                                start=True, stop=True,
                                perf_mode=DRM,
                            )
                            nc.scalar.activation(
                                e_c[:, kt, :], s_ps, AF.Exp,
                                bias=cbt[:, kt:kt + 1], scale=0.0625,
                            )
                    def emit_S_causal(h, qc, cbt=None, bcnt=None):
                        if bcnt is None:
                            bcnt = bcn_sb
                        po = (h % 2) * 64
                        mc = h // 2
                        e_c, e_z = e_tiles[(h, qc)]
                        for qq in range(4):
                            g = 4 * qc + qq
                            if g < 2:
                                continue
                            ckts = list(range(2, g + 1))
                            qql = slice(qc * 512 + qq * P,
                                        qc * 512 + (qq + 1) * P)
                            for p0 in range(0, len(ckts), 4):
                                grp = ckts[p0:p0 + 4]
                                s_z = sps.tile([P, 512], f32, tag="S")
                                for idx, kt in enumerate(grp):
                                    zsl = s_z[:, idx * P:(idx + 1) * P]
                                    diag = kt == g
                                    nc.tensor.matmul(
                                        zsl,
                                        k_sb[po:po + 64, mc,
                                             kt * P:(kt + 1) * P]
                                        .unsqueeze(1)
                                        .broadcast_to([64, 2, P]),
                                        q_sb[po:po + 64, mc, qql]
                                        .unsqueeze(1)
                                        .broadcast_to([64, 2, P]),
                                        start=True, stop=not diag,
                                        perf_mode=DRM,
                                    )
                                    if diag:
                                        # causal triangle via PE: I^T @ tri
                                        # accumulates tri into the diag tile
                                        nc.tensor.matmul(
                                            zsl, id_sb[:], tri_sb[:],
                                            start=False, stop=True,
                                        )
                                dst = e_z[:, p0:p0 + len(grp),
                                          qq * P:(qq + 1) * P]
                                nc.scalar.activation(
                                    dst, s_z[:, 0:len(grp) * P], AF.Exp,
                                    bias=bcnt[:], scale=0.0625,
                                )

                    avd_state = {}

                    def emit_AV(h, qc, qqs):
                        """AV + denominator matmuls for pair (h, qc),
                        query tiles qqs only."""
                        if (h, qc) not in avd_state:
                            avd_state[(h, qc)] = (
                                yps.tile([64, 512], f32, tag="Y",
                                         name=f"y{h}_{qc}"),
                                dps.tile([64, 512], f32, tag="DN",
                                         name=f"dn{h}_{qc}"),
                            )
                        y_ps, dn_ps = avd_state[(h, qc)]
                        e_c, e_z = e_tiles[(h, qc)]
                        ones_bc = ones_dn[:].unsqueeze(1) \
                            .broadcast_to([P, 2, D])
                        for qq in qqs:
                            g = 4 * qc + qq
                            kts = kts_for(qc, qq)
                            npair = (len(kts)) // 2
                            single = (len(kts) % 2) == 1
                            ysl = y_ps[:, qq * P:(qq + 1) * P]
                            dsl = dn_ps[:, qq * P:(qq + 1) * P]
                            for kp in range(npair):
                                if kp == 0:
                                    mv = e_c[:, 0:2, qq * P:(qq + 1) * P]
                                else:
                                    mv = e_z[:, 2 * kp - 2:2 * kp,
                                             qq * P:(qq + 1) * P]
                                st = (kp == 0)
                                sp = (kp == npair - 1) and not single
                                nc.tensor.matmul(
                                    ysl, v_sb[:, 2 * kp:2 * kp + 2, h, :],
                                    mv, start=st, stop=sp, perf_mode=DRM,
                                )
                                nc.tensor.matmul(
                                    dsl, ones_bc, mv,
                                    start=st, stop=sp, perf_mode=DRM,
                                )
                            if single:
                                mv = e_z[:, g - 2, qq * P:(qq + 1) * P]
                                nc.tensor.matmul(
                                    ysl, v_sb[:, g, h, :], mv,
                                    start=False, stop=True,
                                )
                                nc.tensor.matmul(
                                    dsl, ones_dn[:], mv,
                                    start=False, stop=True,
                                )
                    def emit_norm(h, qc):
                        po = (h % 2) * 64
                        mc = h // 2
                        e_tiles.pop((h, qc))
                        y_ps, dn_ps = avd_state.pop((h, qc))
                        # normalize: rb = 1/dn  (bf16), y_sb = y_ps * rb
                        rb = asb.tile([64, 512], bf16, tag="rb")
                        with nc.allow_low_precision(
                            reason="uniform per-row softmax scale; bf16 ok"
                        ):
                            nc.vector.reciprocal(rb, dn_ps[:])
                        qsl = slice(qc * 512, (qc + 1) * 512)
                        if po == 0:
                            nc.vector.tensor_mul(
                                y_sb[0:64, mc, qsl], y_ps[:], rb
                            )
                        else:
                            yt = asb.tile([64, 512], fp8, tag="yt")
                            nc.vector.tensor_mul(yt, y_ps[:], rb)
                            nc.sync.dma_start(y_sb[64:128, mc, qsl], yt)

                    def emit_S(h, qc, cbt=None, bcnt=None):
                        emit_S_cond(h, qc, cbt, bcnt)
                        emit_S_causal(h, qc, cbt, bcnt)

                    def emit_AVD(h, qc):
                        emit_AV(h, qc, range(4))
                        emit_norm(h, qc)

                    def emit_ln2_transp(t, act_evict=False):
                        # transposes borrow the S psum ring (attention S is
                        # quiescent whenever LN2 runs)
                        ln_tile(h_tok, t, ln2_args, act_mean=True,
                                mean_acc=proj_acc.pop(t))
                        transp_tile(hT_hi, t, h_tok, sps, lo8=hT_lo,
                                    act_evict=act_evict)
                        sm_dst = hT_sm[:, :, t * P:(t + 1) * P]
                        sm_src = hT_hi[:, :, t * P:(t + 1) * P]
                        if t % 2 == 0:
                            nc.gpsimd.tensor_scalar_mul(
                                sm_dst, sm_src, float(2.0 ** -5))
                        else:
                            nc.vector.tensor_scalar_mul(
                                sm_dst, sm_src, float(2.0 ** -5))

                    # ---- startup: LN1 t0..3, V t0..3, Q/K n2=0
                    # interleaved so the qc0 sweep can start after the
                    # first head's chunks; LN1 t4..7 and V t4..7 become
                    # in-sweep fillers (DVE-side, ACT is exp-bound there) --
                    for t in range(4):
                        ln_tile(xn_tok, t, ln1_args,
                                act_mean=True,
                                norm_act=(t % 2 == 1))
                        transp_tile(xnT8, t, xn_tok, mmps,
                                    act_evict=(t % 2 == 1))
                    for t in range(4):
                        emit_v(t, 0, act_evict=False)
                        emit_v(t, 1, act_evict=True)
                    for m in range(NKC):
                        qk_chunk(wq_sb, q_sb, "bq", 0, m)
                        qk_chunk(wk_sb, k_sb, "bk", 0, m)

                    # ---- phase plan (act tables: exp and gelu can't share
                    # a table, each swap costs 1.28us of ACT — keep phases
                    # table-pure):
                    #   qc0 sweep (exp)  [fill: QK n2=1, wp DMA]
                    #   proj/LN2 t0..3
                    #   MLP1 half-0 (gelu, PE-dense)
                    #   qc1 sweep (exp)  [fill: MLP2 half-0 — no ACT work]
                    #   proj/LN2 t4..7, MLP1 half-1 (gelu), MLP2 half-1

                    fillers = {}

                    def add_filler(i, fn):
                        fillers.setdefault(i, []).append(fn)

                    # --- qc0 sweep ---
                    # odd heads (DMA-shift eviction path) first: the last
                    # norms then write y_sb directly and proj isn't gated
                    # on a trailing DMA-completion semaphore
                    pairs0 = [(h, 0) for h in range(H) if h % 2 == 1] + \
                             [(h, 0) for h in range(H) if h % 2 == 0]

                    def ln1_late(t):
                        ln_tile(xn_tok, t, ln1_args, act_mean=False,
                                norm_act=False, dve_var=(t % 2 == 1))
                        transp_tile(xnT8, t, xn_tok, mmps,
                                    act_evict=(t % 2 == 0))

                    for t in range(4, NT):
                        add_filler(t - 4, lambda t=t: ln1_late(t))
                        add_filler(t - 3, lambda t=t: emit_v(t, 0))
                        add_filler(t - 3, lambda t=t: emit_v(t, 1))
                    for m in range(NKC):
                        add_filler(5 + m, lambda m=m: qk_chunk(
                            wq_sb, q_sb, "bq", 1, m))
                        add_filler(9 + m if m >= 4 else 9 + m // 2 * 2,
                                   lambda m=m: qk_chunk(
                                       wk_sb, k_sb, "bk", 1, m))
                    add_filler(10, lambda: nc.sync.dma_start(wp_sb[:], wp_re))
                    add_filler(14, lambda: prefetch_w1(0, 0))
                    add_filler(15, lambda: prefetch_w1(1, 0))
                    for i in range(len(pairs0) + 2):
                        if i < len(pairs0):
                            emit_S_cond(*pairs0[i])
                        if i >= 2:
                            emit_AV(*pairs0[i - 2], (0, 1))
                        if i < len(pairs0):
                            emit_S_causal(*pairs0[i])
                        if i >= 2:
                            emit_AV(*pairs0[i - 2], (2, 3))
                            emit_norm(*pairs0[i - 2])
                        for fn in fillers.get(i, ()):
                            fn()

                    # --- bubble fill: qc1 heads 0..2 ungated, proj/LN2
                    # t0..3, MLP1 half-0; later qc1 exps are gated behind
                    # the last gelu via bias-tile indirection so the greedy
                    # per-engine scheduler cannot interleave exp and gelu
                    # (each exp<->gelu transition costs a 1.28us act-table
                    # load). ---
                    emit_S(0, 1)
                    emit_S(1, 1)
                    emit_AVD(0, 1)
                    for t in range(4):
                        emit_proj(t, 0)
                        emit_proj(t, 1)
                    emit_S(2, 1)
                    emit_AVD(1, 1)
                    for t in range(2):
                        emit_ln2_transp(t, act_evict=True)
                    emit_S(3, 1)
                    emit_AVD(2, 1)
                    for t in range(2, 4):
                        emit_ln2_transp(t, act_evict=True)
                    emit_AVD(3, 1)
                    gb_last = None
                    for m in range(NM):
                        gb_last = emit_mlp1(m, 0)

                    # gate tiles: copy of the exp biases that depends on the
                    # last gelu output
                    cb1_sb = cpool.tile([P, 2], f32, tag="cb1")
                    nc.vector.scalar_tensor_tensor(
                        cb1_sb, gb_last[:, 0:2], 0.0, cb_sb[:],
                        op0=OP.mult, op1=OP.add,
                    )
                    bcn1_sb = cb1_sb[:, 0:1]  # col 0 = -BC when counts>0
                    bcn1_ok = cpool.tile([P, 1], f32, tag="bcn1")
                    nc.vector.scalar_tensor_tensor(
                        bcn1_ok, gb_last[:, 0:1], 0.0, bcn_sb[:],
                        op0=OP.mult, op1=OP.add,
                    )

                    # --- gated qc1 sweep (exp) with MLP2 half-0 on PE,
                    # interleaved at tt (1.3us) granularity so freshly
                    # emitted S matmuls don't queue behind a 5us chunk ---
                    prefetch_w2(0, 0)
                    prefetch_w2(1, 0)
                    pieces = mlp2_pieces(0)
                    import itertools

                    def take(k):
                        for fn in itertools.islice(pieces, k):
                            fn()

                    pairs1 = [(h, 1) for h in range(4, H) if h % 2 == 1] + \
                             [(h, 1) for h in range(4, H) if h % 2 == 0]
                    consumed = [0]

                    def ctake(k):
                        if consumed[0] < 24:
                            take(k)
                            consumed[0] += k

                    for i in range(len(pairs1) + 2):
                        if i < len(pairs1):
                            emit_S_cond(*pairs1[i], cbt=cb1_sb,
                                        bcnt=bcn1_ok)
                        if i >= 2:
                            emit_AV(*pairs1[i - 2], (0, 1))
                        ctake(1)
                        if i < len(pairs1):
                            emit_S_causal(*pairs1[i], cbt=cb1_sb,
                                          bcnt=bcn1_ok)
                        ctake(1)
                        if i >= 2:
                            emit_AV(*pairs1[i - 2], (2, 3))
                            emit_norm(*pairs1[i - 2])

                    # ---- tail: proj/LN2 t4..7, MLP1 h1 (gelu), MLP2 h1 ----
                    prefetch_w1(0, 1)
                    prefetch_w1(1, 1)
                    for t in range(4, NT):
                        emit_proj(t, 0)
                        emit_proj(t, 1)
                        take(2)
                        emit_ln2_transp(t, act_evict=True)
                    take(100)
                    for m in range(NM):
                        emit_mlp1(m, 1)
                        if m == 24:
                            prefetch_w2(0, 1)
                    for n8 in range(NKC):
                        emit_mlp2_chunk(n8, 1)
                # end attention scope

    nc.compile()
    return nc


def _host_aux(cond_mask):
    counts = np.asarray(cond_mask).sum(axis=-1).astype(np.int64)  # [B]
    cbias = []
    for b in range(B):
        vec = np.full(COND_LEN, -BC, np.float32)
        vec[counts[b]:] = NEG
        cbias.append(np.ascontiguousarray(vec.reshape(2, P).T))
    kk = np.arange(P)[:, None]
    qq = np.arange(P)[None, :]
    tri = np.where(qq >= kk, 0.0, NEG).astype(BF16)
    ident = np.eye(P, dtype=BF16)
    return cbias, tri, ident


def kernel(**inputs):
    from concourse.bass_utils import run_bass_kernel_spmd

    x = np.asarray(inputs["x"], np.float32)
    assert x.shape == (B, T, C)
    assert int(inputs["cond_len"]) == COND_LEN
    assert int(inputs["token_len"]) == TOKEN_LEN

    f32 = np.float32
    Wq, Wk, Wv, Wp = (np.asarray(inputs[k], f32) for k in ("Wq", "Wk", "Wv", "Wp"))
    W1, W2 = np.asarray(inputs["W1"], f32), np.asarray(inputs["W2"], f32)
    bq, bk, bv, bp = (np.asarray(inputs[k], f32) for k in ("bq", "bk", "bv", "bp"))
    b1, b2 = np.asarray(inputs["b1"], f32), np.asarray(inputs["b2"], f32)
    g1, o1 = np.asarray(inputs["ln1_g"], f32), np.asarray(inputs["ln1_b"], f32)
    g2, o2 = np.asarray(inputs["ln2_g"], f32), np.asarray(inputs["ln2_b"], f32)

    flags = (
        bool(bq.any() or bk.any()),
        bool(bv.any()),
        bool(bp.any()),
        bool(b1.any()),
        bool(b2.any()),
        bool((g1 != 1).any() or o1.any()),
        bool((g2 != 1).any() or o2.any()),
    )
    if flags not in _BUILD_CACHE:
        _BUILD_CACHE[flags] = _build(flags)
    nc = _BUILD_CACHE[flags]
    qk_bias, v_bias, p_bias, b1_bias, b2_bias, ln1_aff, ln2_aff = flags

    cbias, tri, ident = _host_aux(inputs["cond_mask"])
    w1h = W1.astype(F8)
    w1l = ((W1 - w1h.astype(f32)) * 32.0).astype(F8)
    w2h = W2.astype(F8)
    w2l = ((W2 - w2h.astype(f32)) * 32.0).astype(F8)

    def lay1(w):  # [C, FF] -> [P, NM, NKC, P] chunk-contiguous
        return np.ascontiguousarray(
            w.reshape(NKC, P, NM, P).transpose(1, 2, 0, 3))

    def lay2(w):  # [FF, C] -> [P, NKC, NM, P]
        return np.ascontiguousarray(
            w.reshape(NM, P, NKC, P).transpose(1, 2, 0, 3))

    shared = {
        "wq": Wq.astype(F8), "wk": Wk.astype(F8),
        "wv": Wv.astype(F8), "wp": Wp.astype(F8),
        "w1h": lay1(w1h), "w1l": lay1(w1l),
        "w2h": lay2(w2h), "w2l": lay2(w2l),
        "tri": tri, "ident": ident,
    }
    if qk_bias:
        shared["bq"] = np.ascontiguousarray(bq.reshape(NKC, P).T)
        shared["bk"] = np.ascontiguousarray(bk.reshape(NKC, P).T)
    if v_bias:
        shared["bv"] = bv.reshape(1, C).astype(BF16)
    if p_bias:
        shared["bp"] = bp.reshape(1, C).astype(BF16)
    if b1_bias:
        shared["b1"] = np.ascontiguousarray(b1.reshape(NM, P).T)
    if b2_bias:
        shared["b2"] = b2.reshape(1, C).astype(BF16)
    if ln1_aff:
        shared["g1"] = np.broadcast_to(g1, (P, C)).copy()
        shared["o1"] = np.broadcast_to(o1, (P, C)).copy()
    if ln2_aff:
        shared["g2"] = np.broadcast_to(g2, (P, C)).copy()
        shared["o2"] = np.broadcast_to(o2, (P, C)).copy()

    in_maps = [dict(shared, x=x[b].astype(BF16), cbias=cbias[b])
               for b in range(B)]
    try:
        res = run_bass_kernel_spmd(nc, in_maps, list(range(B)),
                                   trace=kernel._trace)
    except ModuleNotFoundError:
        res = run_bass_kernel_spmd(nc, in_maps, list(range(B)), trace=False)
    kernel._last_results = res
    out = np.stack([res.results[b]["out"] for b in range(B)], axis=0)
    return out.astype(np.float32)


kernel._trace = False
kernel._last_results = None
